# revision 1
# baseline (speedup 1.0000x reference)
"""Trainium2 Bass kernel for the CECL contrastive loss (nn_CeclLossModule).

Strategy (8 NeuronCores, SPMD):
  - N = B*A = 6400 rows, D = 256. Core c owns global rows [800c, 800c+800)
    (padded to 896 = 7*128; pad rows wrap around and are discarded on host).
  - Each core receives the full embedding/time arrays ROTATED by 800c so
    that (a) its own rows are the first 896 columns, letting the lhsT tiles
    be slices of the shared normalized-transposed matrix FT, and (b) the
    "same-sample" 8-wide block-diagonal of its row-block always sits at
    local columns [128t, 128t+128) for row-tile t, identical on all cores
    (required because all cores run one identical program).
  - Per row-tile of 128 rows: z = F_rows @ F.T via fp32r matmuls into PSUM.
    Valid-negative mask nov[i,j] = [max(sf_i,sf_j) > min(ef_i,ef_j)] is
    built from two tensor_scalar ops (GPSIMD) + one fused
    tensor_tensor_reduce (DVE) that also accumulates the per-row count.
    zm = z + BIG*nov, and one ScalarE Softplus with fused scale/bias
    (bias' = bias - scale*BIG) and fused row-sum accumulation yields
    sum_j nov * softplus(scale*z+bias) directly: masked (overlapping)
    elements see softplus(-large) = 0.
  - The positive-pair terms and the in-group corrections are handled on the
    static 128x128 diagonal block per row-tile with three small fused
    reduces against constant EQ / (EQ - I) patterns.
  - Per-row nll = rowsum / count computed on-device; host gathers the 8x896
    vectors, drops padding, and takes the mean.
"""

import numpy as np

N = 6400
D = 256
A = 8
NCORES = 8
RPC = 800          # rows per core
RT = 7             # row tiles per core (896 rows, 96 pad)
RTP = RT * 128     # 896
CTW = 512          # col tile width
NCT = 13           # 12*512 + 256
BIG = 2048.0

_cached = {}


def build():
    """Build the full Bass program. Returns nc."""
    import concourse.bass as bass
    import concourse.bacc as bacc
    import concourse.tile as tile
    from concourse import mybir
    from contextlib import ExitStack

    f32 = mybir.dt.float32
    f32r = mybir.dt.float32r
    ALU = mybir.AluOpType
    ACTF = mybir.ActivationFunctionType
    AX = mybir.AxisListType

    nc = bacc.Bacc("TRN2", target_bir_lowering=False)
    ecols = nc.declare_dram_parameter("ecols", [N, D], f32, isOutput=False)
    sfc = nc.declare_dram_parameter("sfc", [N], f32, isOutput=False)
    efc = nc.declare_dram_parameter("efc", [N], f32, isOutput=False)
    eqcd = nc.declare_dram_parameter("eqc", [128, 128], f32, isOutput=False)
    poscd = nc.declare_dram_parameter("posc", [128, 128], f32, isOutput=False)
    idnd = nc.declare_dram_parameter("idn", [128, 128], f32, isOutput=False)
    scld = nc.declare_dram_parameter("scl", [1], f32, isOutput=False)
    biad = nc.declare_dram_parameter("bia", [1], f32, isOutput=False)
    nlld = nc.declare_dram_parameter("nll", [RTP], f32, isOutput=True)

    with ExitStack() as ctx:
        tc = ctx.enter_context(tile.TileContext(nc))

        singles = ctx.enter_context(tc.tile_pool(name="singles", bufs=1))
        sspool = ctx.enter_context(tc.tile_pool(name="ss", bufs=3))
        smallpool = ctx.enter_context(tc.tile_pool(name="small", bufs=4))
        partpool = ctx.enter_context(tc.tile_pool(name="part", bufs=2))

        # ----- constants / scalars -----
        eqc_t = singles.tile([128, 128], f32)
        nc.sync.dma_start(out=eqc_t, in_=eqcd[:, :])
        posc_t = singles.tile([128, 128], f32)
        nc.sync.dma_start(out=posc_t, in_=poscd[:, :])
        idn_t = singles.tile([128, 128], f32)
        nc.sync.dma_start(out=idn_t, in_=idnd[:, :])

        scl_t = singles.tile([128, 1], f32)
        nc.gpsimd.dma_start(out=scl_t, in_=scld[:].to_broadcast([128, 1]))
        bia_t = singles.tile([128, 1], f32)
        nc.gpsimd.dma_start(out=bia_t, in_=biad[:].to_broadcast([128, 1]))
        # bias_eff = bias - BIG*scale ; nscl = -scale ; nbia = -bias
        bias_eff = singles.tile([128, 1], f32)
        nc.vector.scalar_tensor_tensor(
            out=bias_eff, in0=scl_t, scalar=-BIG, in1=bia_t,
            op0=ALU.mult, op1=ALU.add)
        nscl_t = singles.tile([128, 1], f32)
        nc.vector.tensor_scalar_mul(nscl_t, scl_t, -1.0)
        nbia_t = singles.tile([128, 1], f32)
        nc.vector.tensor_scalar_mul(nbia_t, bia_t, -1.0)

        # per-row start/end times: sfp[p, t] = sf[128t + p]
        sfp = singles.tile([128, 50], f32)
        nc.sync.dma_start(out=sfp, in_=sfc.rearrange("(t p) -> p t", p=128))
        efp = singles.tile([128, 50], f32)
        nc.sync.dma_start(out=efp, in_=efc.rearrange("(t p) -> p t", p=128))

        # ----- phase 1a: broadcast sf/ef to all 128 partitions (DMA) -----
        SFB = singles.tile([128, N], f32)
        EFB = singles.tile([128, N], f32)
        import concourse.bass as bass_mod
        for ct in range(NCT):
            off = ct * CTW
            w = min(CTW, N - off)
            for src, dst in ((sfc, SFB), (efc, EFB)):
                sl = src[off:off + w]
                bcast = bass_mod.AP(tensor=sl.tensor, offset=sl.offset,
                                    ap=[[0, 128]] + list(sl.ap))
                nc.gpsimd.dma_start(out=dst[:, off:off + w], in_=bcast)

        # ----- phase 1b: normalize embeddings + transpose into FT -----
        # FT flat layout: chunk k of row-tile t lives at cols 256*t + 128*k.
        FT = singles.tile([128, 2 * N], f32r)
        ssb = singles.tile([128, 50], f32)
        invb = singles.tile([128, 50], f32)
        HB = 25  # tiles per half-batch
        with ExitStack() as p1:
            ebpool = p1.enter_context(tc.tile_pool(name="eb", bufs=1))
            fpool = p1.enter_context(tc.tile_pool(name="f", bufs=3))
            tp_psum = p1.enter_context(
                tc.tile_pool(name="tpp", bufs=2, space="PSUM"))
            for half in range(2):
                base = HB * half
                Ebuf = ebpool.tile([128, HB * D], f32, tag="eb")
                nc.sync.dma_start(
                    out=Ebuf,
                    in_=ecols[128 * base:128 * (base + HB), :].rearrange(
                        "(t p) d -> p t d", p=128))
                for j in range(HB):
                    t = base + j
                    sq = fpool.tile([128, D], f32, tag="sq")
                    nc.scalar.activation(
                        sq, Ebuf[:, j * D:(j + 1) * D], ACTF.Square,
                        accum_out=ssb[:, t:t + 1])
                lnb = sspool.tile([128, HB], f32, tag="lnb")
                nc.scalar.activation(lnb, ssb[:, base:base + HB], ACTF.Ln)
                nc.scalar.activation(invb[:, base:base + HB], lnb,
                                     ACTF.Exp, scale=-0.5)
                for j2 in range(0, HB, 2):
                    npair = min(2, HB - j2)
                    tp = tp_psum.tile([128, 512], f32, tag="tp")
                    for jj in range(npair):
                        j = j2 + jj
                        t = base + j
                        fn = fpool.tile([128, D], f32, tag="fn")
                        nc.vector.tensor_scalar_mul(
                            fn, Ebuf[:, j * D:(j + 1) * D],
                            invb[:, t:t + 1])
                        for k in range(2):
                            nc.tensor.transpose(
                                tp[:, 256 * jj + 128 * k:
                                   256 * jj + 128 * k + 128],
                                fn[:, 128 * k:128 * k + 128], idn_t)
                    nc.vector.tensor_copy(
                        FT[:, 256 * (base + j2):
                           256 * (base + j2) + 256 * npair],
                        tp[:, :256 * npair])

        # ----- phase 2: bulk row-block loss -----
        # Supertiles of 2048 cols; diag block (local cols 128*rt) is always
        # inside supertile 0. ACT work is grouped per row-tile by function
        # (all Exp, then all Ln) to avoid per-op ACT table reloads.
        FTv = FT.rearrange("p (t k c) -> p t k c", k=2, c=128)
        SW = [2048, 2048, 2048, 256]
        SOFF = [0, 2048, 4096, 6144]
        NS = 4
        nllb = singles.tile([128, RT], f32)
        exbpool = ctx.enter_context(tc.tile_pool(name="exb", bufs=1))
        btpool = ctx.enter_context(tc.tile_pool(name="bt", bufs=1))
        zmpool = ctx.enter_context(tc.tile_pool(name="zmp", bufs=2))
        spmpool = ctx.enter_context(tc.tile_pool(name="spmp", bufs=1))
        zw_psum = ctx.enter_context(
            tc.tile_pool(name="zw", bufs=2, space="PSUM"))
        for rt in range(RT):
            sf_i = sfp[:, rt:rt + 1]
            ef_i = efp[:, rt:rt + 1]
            sub = 128 * rt
            EXbuf = exbpool.tile([128, N + 128], f32, tag="exb")
            cparts = partpool.tile([128, 2 * NS], f32, tag="cp")
            s1parts = partpool.tile([128, NS], f32, tag="s1p")
            negc = smallpool.tile([128, 1], f32, tag="negc")
            cntc = smallpool.tile([128, 1], f32, tag="cntc")
            posc_acc = smallpool.tile([128, 1], f32, tag="posa")

            zws = []
            for s in range(NS):
                off, w = SOFF[s], SW[s]
                zw = zw_psum.tile([128, 2048], f32, tag="z",
                                  name=f"zw{rt}_{s}")
                zws.append(zw)
                for k in range(2):
                    lhsT = FT[:, 256 * rt + 128 * k:256 * rt + 128 * k + 128]
                    for b in range(0, w, 512):
                        bw = min(512, w - b)
                        nt = bw // 128
                        t0 = (off + b) // 128
                        rhs = FTv[:, t0:t0 + nt, k, :]
                        nc.tensor.matmul(
                            zw[:, b:b + bw], lhsT=lhsT, rhs=rhs,
                            start=(k == 0), stop=(k == 1))

                # mask chain (all DVE) + Exp (grouped)
                bt1 = btpool.tile([128, 2048], f32, tag="bt1")
                nc.vector.tensor_scalar(
                    out=bt1[:, :w], in0=SFB[:, off:off + w],
                    scalar1=ef_i, scalar2=0.0,
                    op0=ALU.is_gt, op1=ALU.add,
                    accum_out=cparts[:, s:s + 1])
                bt2 = btpool.tile([128, 2048], f32, tag="bt2")
                nc.vector.tensor_scalar(
                    out=bt2[:, :w], in0=EFB[:, off:off + w],
                    scalar1=sf_i, scalar2=0.0,
                    op0=ALU.is_lt, op1=ALU.add,
                    accum_out=cparts[:, NS + s:NS + s + 1])
                bt12 = btpool.tile([128, 2048], f32, tag="bt12")
                nc.vector.tensor_tensor(
                    out=bt12[:, :w], in0=bt1[:, :w], in1=bt2[:, :w],
                    op=ALU.add)
                zm = zmpool.tile([128, 2048], f32, tag="zm")
                nc.vector.scalar_tensor_tensor(
                    out=zm[:, :w], in0=bt12[:, :w], scalar=BIG,
                    in1=zw[:, :w], op0=ALU.mult, op1=ALU.add)
                nc.scalar.activation(
                    EXbuf[:, off:off + w], zm[:, :w], ACTF.Exp,
                    bias=bias_eff, scale=scl_t)
                if s == 0:
                    # diag-block exp(-(scale*z+bias)) from raw PSUM z
                    nc.scalar.activation(
                        EXbuf[:, N:N + 128], zw[:, sub:sub + 128],
                        ACTF.Exp, bias=nbia_t, scale=nscl_t)
                    scr2 = smallpool.tile([128, 128], f32, tag="scr2")
                    nc.vector.tensor_tensor(
                        out=scr2, in0=bt12[:, sub:sub + 128], in1=eqc_t,
                        op=ALU.mult)
                    scr2b = smallpool.tile([128, 128], f32, tag="scr2b")
                    nc.vector.tensor_scalar(
                        out=scr2b, in0=scr2, scalar1=1.0, scalar2=0.0,
                        op0=ALU.mult, op1=ALU.add, accum_out=cntc)

            # Ln group
            for s in range(NS):
                off, w = SOFF[s], SW[s]
                spm = spmpool.tile([128, 2048], f32, tag="spm")
                nc.scalar.activation(
                    spm[:, :w], EXbuf[:, off:off + w], ACTF.Ln, bias=1.0,
                    accum_out=s1parts[:, s:s + 1])
                if s == 0:
                    scr = smallpool.tile([128, 128], f32, tag="scr")
                    nc.vector.tensor_tensor(
                        out=scr, in0=spm[:, sub:sub + 128], in1=eqc_t,
                        op=ALU.mult)
                    scrb = smallpool.tile([128, 128], f32, tag="scrb")
                    nc.vector.tensor_scalar(
                        out=scrb, in0=scr, scalar1=1.0, scalar2=0.0,
                        op0=ALU.mult, op1=ALU.add, accum_out=negc)
            spn = smallpool.tile([128, 128], f32, tag="spn")
            nc.scalar.activation(spn, EXbuf[:, N:N + 128], ACTF.Ln, bias=1.0)
            scr3 = smallpool.tile([128, 128], f32, tag="scr3")
            nc.vector.scalar_tensor_tensor(
                out=scr3, in0=posc_t, scalar=1.0, in1=spn,
                op0=ALU.mult, op1=ALU.mult, accum_out=posc_acc)

            # ----- per-row finalization -----
            s1 = smallpool.tile([128, 1], f32, tag="s1")
            nc.vector.reduce_sum(s1, s1parts, axis=AX.X)
            csum = smallpool.tile([128, 1], f32, tag="csum")
            nc.vector.reduce_sum(csum, cparts, axis=AX.X)
            # rowsum = s1 - negc + posc_acc
            rowsum = smallpool.tile([128, 1], f32, tag="rows")
            nc.vector.scalar_tensor_tensor(
                out=rowsum, in0=s1, scalar=negc, in1=posc_acc,
                op0=ALU.subtract, op1=ALU.add)
            # cnt = csum - cntc + 7
            cnt = smallpool.tile([128, 1], f32, tag="cnt")
            nc.vector.scalar_tensor_tensor(
                out=cnt, in0=csum, scalar=1.0, in1=cntc,
                op0=ALU.mult, op1=ALU.subtract)
            cnt7 = smallpool.tile([128, 1], f32, tag="cnt7")
            nc.vector.tensor_scalar_add(cnt7, cnt, float(A - 1))
            icnt = smallpool.tile([128, 1], f32, tag="icnt")
            nc.vector.reciprocal(icnt, cnt7)
            nc.vector.tensor_tensor(
                out=nllb[:, rt:rt + 1], in0=rowsum, in1=icnt, op=ALU.mult)

        for rt in range(RT):
            nc.sync.dma_start(out=nlld[128 * rt:128 * rt + 128],
                              in_=nllb[:, rt:rt + 1])
    nc.compile()
    return nc


def _get_nc():
    if "nc" not in _cached:
        _cached["nc"] = build()
    return _cached["nc"]


def kernel(embeddings, start_times, end_times, logit_scale, logit_bias):
    from concourse.bass_utils import run_bass_kernel_spmd

    emb = np.ascontiguousarray(np.asarray(embeddings), dtype=np.float32).reshape(N, D)
    sf = np.ascontiguousarray(np.asarray(start_times), dtype=np.float32).reshape(N)
    ef = np.ascontiguousarray(np.asarray(end_times), dtype=np.float32).reshape(N)
    scl = np.asarray(logit_scale, dtype=np.float32).reshape(1)
    bia = np.asarray(logit_bias, dtype=np.float32).reshape(1)

    gid = np.arange(128) // A
    eqc = (gid[:, None] == gid[None, :]).astype(np.float32)
    posc = eqc - np.eye(128, dtype=np.float32)
    idn = np.eye(128, dtype=np.float32)

    in_maps = []
    for c in range(NCORES):
        rot = np.roll(np.arange(N), -RPC * c)
        in_maps.append({
            "ecols": np.ascontiguousarray(emb[rot]),
            "sfc": np.ascontiguousarray(sf[rot]),
            "efc": np.ascontiguousarray(ef[rot]),
            "eqc": eqc, "posc": posc, "idn": idn,
            "scl": scl, "bia": bia,
        })

    nc = _get_nc()
    res = run_bass_kernel_spmd(nc, in_maps, list(range(NCORES)), **_run_opts)
    _cached["last_result"] = res
    nll = np.concatenate([res.results[c]["nll"][:RPC] for c in range(NCORES)])
    return np.float32(nll.mean())


# test-harness knob: test.py sets _run_opts["trace"] = True to get exec_time_ns
_run_opts = {}

